# revision 3
# baseline (speedup 1.0000x reference)
"""AdjacencyProjector kernel for 8 Trainium2 NeuronCores.

score[b, i, j] = E[b, i] . W[0, :D]  +  E[b, j] . W[0, D:]

B=4, N=4096, D=128. Output (4, 4096, 4096) f32 = 256MB -> memory (write)
bound. Sharding: 8 cores x (batch, row-half): core k computes rows
[h*2048, (h+1)*2048) of batch b where b = k//2, h = k%2. Each core gets
the full batch E (2MB) to compute the column term bvec, plus its own row
half (1MB) for the row term avec, and writes a 32MB output shard.
"""

import sys

sys.path.insert(0, "/opt/trn_rl_repo")

import numpy as np

B, N, D = 4, 4096, 128
P = 128
ROWS_PER_CORE = N // 2          # 2048
NT = N // P                     # 32 column chunks per batch
NR = ROWS_PER_CORE // P         # 16 row blocks per core
N_CORES = 8

_CACHE = {}


def _build_nc():
    import concourse.bacc as bacc
    import concourse.bass as bass
    import concourse.mybir as mybir
    from concourse.tile import TileContext
    from concourse.masks import make_identity

    f32 = mybir.dt.float32
    nc = bacc.Bacc("TRN2", num_devices=N_CORES)

    eb_d = nc.declare_dram_parameter("Eb", [N, D], f32, isOutput=False)
    er_d = nc.declare_dram_parameter("Er", [ROWS_PER_CORE, D], f32, isOutput=False)
    w_d = nc.declare_dram_parameter("W", [1, 2 * D], f32, isOutput=False)
    out_d = nc.declare_dram_parameter("out", [ROWS_PER_CORE, N], f32, isOutput=True)

    def bcast_free(ap, n, at=1):
        # insert a stride-0 dim of size n at free position `at`
        return bass.AP(
            tensor=ap.tensor,
            offset=ap.offset,
            ap=ap.ap[:at] + [[0, n]] + ap.ap[at:],
        )

    with TileContext(nc) as tc:
        with (
            tc.tile_pool(name="consts", bufs=1) as consts,
            tc.tile_pool(name="work", bufs=1) as work,
            tc.tile_pool(name="psum", bufs=2, space="PSUM") as psum,
            tc.tile_pool(name="outp", bufs=4) as outp,
        ):
            ident = consts.tile([P, P], f32)
            make_identity(nc, ident)
            ones = consts.tile([1, P], f32)
            nc.vector.memset(ones, 1.0)

            wi_rep = consts.tile([P, D], f32)
            nc.gpsimd.dma_start(
                out=wi_rep, in_=w_d.ap()[0:1, 0:D].partition_broadcast(P)
            )
            wj_rep = consts.tile([P, D], f32)
            nc.gpsimd.dma_start(
                out=wj_rep, in_=w_d.ap()[0:1, D : 2 * D].partition_broadcast(P)
            )

            # ---- column term: bvec[j] = Eb[j] . wj, j in [0, N) ----
            eb = work.tile([P, NT, D], f32)
            nc.sync.dma_start(
                out=eb, in_=eb_d.ap().rearrange("(t p) d -> p t d", p=P)
            )
            prod = work.tile([P, NT, D], f32)
            nc.vector.tensor_mul(
                out=prod, in0=eb, in1=bcast_free(wj_rep[:], NT)
            )
            bcols = work.tile([P, NT], f32)
            nc.vector.tensor_reduce(
                out=bcols, in_=prod, axis=mybir.AxisListType.X, op=mybir.AluOpType.add
            )

            # ---- row term: avec[i] = Er[i] . wi, i in [0, 2048) ----
            er = work.tile([P, NR, D], f32)
            nc.sync.dma_start(
                out=er, in_=er_d.ap().rearrange("(r p) d -> p r d", p=P)
            )
            prod2 = work.tile([P, NR, D], f32)
            nc.vector.tensor_mul(
                out=prod2, in0=er, in1=bcast_free(wi_rep[:], NR)
            )
            acols = work.tile([P, NR], f32)
            nc.vector.tensor_reduce(
                out=acols, in_=prod2, axis=mybir.AxisListType.X, op=mybir.AluOpType.add
            )

            # ---- bvec columns -> chunk-rows on partitions (bt[t, p] = bvec[t*128+p])
            btp = psum.tile([NT, P], f32)
            nc.tensor.transpose(btp[:], bcols[:], ident[:])
            bt = work.tile([NT, P], f32)
            nc.scalar.copy(out=bt, in_=btp)

            # collapse to a single-partition row: brow[0, j] = bvec[j]
            brow = work.tile([1, N], f32)
            nc.sync.dma_start(out=brow[:], in_=bt[:])

            # ---- brep[p, j] = bvec[j] for all p: rank-1 matmuls ones^T x chunk
            brep = work.tile([P, N], f32)
            for g in range(8):
                pb = psum.tile([P, 512], f32, tag="pb")
                nc.tensor.matmul(
                    pb[:],
                    ones[:],
                    brow[0:1, g * 512 : (g + 1) * 512],
                    start=True,
                    stop=True,
                )
                nc.vector.tensor_copy(
                    out=brep[:, g * 512 : (g + 1) * 512], in_=pb
                )

            # ---- output tiles: out[r*128+p, j] = brep[p, j] + avec[r*128+p]
            for r in range(NR):
                ot = outp.tile([P, N], f32, tag="ot")
                if r % 2 == 0:
                    nc.vector.tensor_scalar_add(ot[:], brep[:], acols[:, r : r + 1])
                else:
                    nc.scalar.add(ot[:], brep[:], acols[:, r : r + 1])
                nc.sync.dma_start(out=out_d.ap()[r * P : (r + 1) * P, :], in_=ot)

    nc.compile()
    return nc


def _get_nc():
    if "nc" not in _CACHE:
        _CACHE["nc"] = _build_nc()
    return _CACHE["nc"]


def _run(E, W, trace=False, tmpdir=None):
    from concourse.bass_utils import run_bass_kernel_spmd

    E = np.asarray(E, dtype=np.float32)
    W = np.asarray(W, dtype=np.float32)
    nc = _get_nc()

    in_maps = []
    for k in range(N_CORES):
        b, h = k // 2, k % 2
        in_maps.append(
            {
                "Eb": np.ascontiguousarray(E[b]),
                "Er": np.ascontiguousarray(
                    E[b, h * ROWS_PER_CORE : (h + 1) * ROWS_PER_CORE]
                ),
                "W": W,
            }
        )
    res = run_bass_kernel_spmd(
        nc, in_maps, core_ids=list(range(N_CORES)), trace=trace, tmpdir=tmpdir
    )
    out = np.empty((B, N, N), dtype=np.float32)
    for k in range(N_CORES):
        b, h = k // 2, k % 2
        out[b, h * ROWS_PER_CORE : (h + 1) * ROWS_PER_CORE, :] = res.results[k]["out"]
    return out, res


def kernel(E, W):
    out, _ = _run(E, W)
    return out
